# revision 31
# baseline (speedup 1.0000x reference)
"""GAT (graph attention network) Trainium2 Bass kernel.

Strategy (8 NeuronCores): shard (batch, node-rows) -- core c handles batch
c//4 and output rows i0 = (c%4)*512 .. i0+512.  All compute is done in the
"transposed" layout P^T[j, i] (j = attended-over node on partitions, i =
output node on free axis), which makes every matmul's contraction dim land
on partitions with zero on-device transposes of big tensors:

  phase 1 (init_weight_graph):  e1 = adj @ adj^T and e2 = b1 @ adj^T as
    fp8 DoubleRow matmuls (exact: 0/1 operands, fp32 PSUM accumulation);
    adjw is a linear combination  w3*[r0*eye + r1*adj + r2*b1 + b2]
    (monotone nesting adj <= b1 <= b2 makes the e_graph increments exact
    differences), materialized via a fused custom DVE op with masked
    entries driven to a large negative value.
  phase 2 (attention):   scores s = leaky((f1_i + f2_j) * adjww) via one
    fused custom DVE op per tile; p = exp(s) on ScalarE (masked -> exp(-1e2)
    -> 0); numerator and softmax denominator come from a single PE matmul
    with an all-ones column appended to h.
  f1/f2 are computed as x @ (W @ a) (tiny va vectors) so no h^T is needed.
  The h_out needed by the output layer is the only cross-core data: a
  128KB AllGather within each batch's 4-core group.

Self-contained: hardcodes shapes from the problem spec.
"""

import os
import sys

import numpy as np

if "/opt/trn_rl_repo" not in sys.path:
    sys.path.insert(0, "/opt/trn_rl_repo")

import ml_dtypes  # noqa: E402

F8NP = ml_dtypes.float8_e4m3

# problem shapes
B, N, NFEAT, NHID, NHEADS, NCLS = 2, 2048, 256, 64, 8, 16
P = 128            # SBUF partitions
IB = 512           # i-rows per core
JC = N // P        # 16 j-chunks
CC = NFEAT // P    # 2 feature chunks
NCORES = 8
GROUPS = 2         # head groups of 4 for h packing
ALPHA = 0.2
BIG = 4096.0
NEGS = -100.0      # masked score fed to exp (exp(-100) == 0 in fp32)

_S2 = 2.0 * (3 / 2.0) ** 2
W1 = float(np.exp(-1.0 / _S2))
W2 = float(np.exp(-4.0 / _S2))
W3 = float(np.exp(-9.0 / _S2))
R0 = (1.0 - W1) / W3
R1 = (W1 - W2) / W3
R2 = (W2 - W3) / W3

_CACHE: dict = {}


# --------------------------------------------------------------------------- #
# custom DVE ops
# --------------------------------------------------------------------------- #
def _register_custom_ops():
    """Register the two fused DVE ops (idempotent, append-only)."""
    if "ops" in _CACHE:
        return _CACHE["ops"]
    from concourse import dve_ops
    from concourse.dve_ops import DveOp
    from concourse.dve_spec import C0, C1, C2, One, Spec, Src0, Src1, Zero, maxx, minn, select
    from concourse.dve_table_gen import dve_ver_for

    def _make(name, spec, ver):
        existing = {op.name: op for op in dve_ops.OPS}
        if name in existing:
            return existing[name]
        op = DveOp(name, spec, subdim=False, uops_sha={})
        idx = len(dve_ops.OPS)
        dve_ops.OPS.append(op)
        dve_ops.CUSTOM_DVE_SPECS[name] = spec
        dve_ops._SUB_OPCODE_FOR_NAME[name] = dve_ops._CUSTOM_DVE_ROW_BASE + idx
        try:
            op.compile(ver)
        except ValueError as e:  # parse the real sha out of the drift error
            import re

            m = re.search(r":\s*(\w+)\s*≠", str(e))
            if not m:
                raise
            op = DveOp(name, spec, subdim=False, uops_sha={ver: m.group(1)})
            dve_ops.OPS[idx] = op
        op.compile(ver)
        return op

    ver = dve_ver_for("TRN2")
    _y = (Src0 + C0) * Src1
    leaky = _make(
        "GAT_LEAKY_ATT",
        Spec(
            body=select(Src1 > Zero, maxx(_y, _y * C2), C1),
            reference=lambda in0, in1, s0, s1, imm2: np.where(
                in1 > 0,
                np.maximum((in0 + s0) * in1, (in0 + s0) * in1 * imm2),
                s1,
            ).astype(np.float32),
        ),
        ver,
    )
    msab = _make(
        "GAT_MIN_SCALE_ADD",
        Spec(
            body=minn(Src0, One) * C0 + Src1 + C1,
            reference=lambda in0, in1, s0, s1, imm2: (
                np.minimum(in0.astype(np.float32), 1.0) * s0 + in1 + s1
            ).astype(np.float32),
        ),
        ver,
    )
    _CACHE["ops"] = (leaky, msab)
    return _CACHE["ops"]


# --------------------------------------------------------------------------- #
# device program
# --------------------------------------------------------------------------- #
def _build_nc():
    if "nc" in _CACHE:
        return _CACHE["nc"]
    from concourse import bacc, bass, mybir, tile

    LEAKY, MSAB = _register_custom_ops()
    f32 = mybir.dt.float32
    f16 = mybir.dt.float16
    bf16 = mybir.dt.bfloat16
    f8 = mybir.dt.float8e4
    AF = mybir.ActivationFunctionType
    AL = mybir.AluOpType

    nc = bacc.Bacc("TRN2", target_bir_lowering=False, debug=False, num_devices=NCORES)

    # ---------------- external I/O ----------------
    d_tadj = nc.dram_tensor("t_adj", [P, JC, N], f8, kind="ExternalInput").ap()
    d_tadjc = nc.dram_tensor("t_adjc", [P, JC, IB], f8, kind="ExternalInput").ap()
    d_tbase = nc.dram_tensor("t_base", [P, JC, IB], f32, kind="ExternalInput").ap()
    d_xt = nc.dram_tensor("xt", [P, CC, N], f16, kind="ExternalInput").ap()
    d_xtc = nc.dram_tensor("xtc", [P, CC, IB], f16, kind="ExternalInput").ap()
    d_w4 = nc.dram_tensor("w4", [P, CC, GROUPS, 4 * NHID], f16, kind="ExternalInput").ap()
    d_wst = nc.dram_tensor("wst", [NHID, NHEADS * CC, P], f32, kind="ExternalInput").ap()
    d_avec = nc.dram_tensor("avec", [NHID, 2 * NHEADS], f32, kind="ExternalInput").ap()
    d_wout = nc.dram_tensor("wout", [P, 4, NCLS], f32, kind="ExternalInput").ap()
    d_aovec = nc.dram_tensor("aovec", [NCLS, 2], f32, kind="ExternalInput").ap()
    d_ident = nc.dram_tensor("ident", [P, P], f32, kind="ExternalInput").ap()
    d_sel8 = nc.dram_tensor("sel8", [NHEADS, NHEADS * P], f32, kind="ExternalInput").ap()
    d_out = nc.dram_tensor("out", [IB, NCLS], f32, kind="ExternalOutput").ap()
    DBG = os.environ.get("GAT_DBG", "0") == "1"
    if DBG:
        d_dbg_adjww = nc.dram_tensor("dbg_adjww", [P, JC, IB], f32, kind="ExternalOutput").ap()
        d_dbg_f1 = nc.dram_tensor("dbg_f1", [NHEADS, IB], f32, kind="ExternalOutput").ap()
        d_dbg_f2 = nc.dram_tensor("dbg_f2", [P, P], f32, kind="ExternalOutput").ap()
        d_dbg_haug = nc.dram_tensor("dbg_haug", [P, GROUPS, JC, 4, NHID + 1], f32, kind="ExternalOutput").ap()
        d_dbg_xh1 = nc.dram_tensor("dbg_xh1", [P, 4, IB], f32, kind="ExternalOutput").ap()
        d_dbg_hloc = nc.dram_tensor("dbg_hloc", [NCLS, IB], f32, kind="ExternalOutput").ap()
        d_dbg_hT = nc.dram_tensor("dbg_hT", [NCLS, N], f32, kind="ExternalOutput").ap()

    with tile.TileContext(nc) as tc:
        with (
            tc.tile_pool(name="persist", bufs=1) as pp,
            tc.tile_pool(name="recycle", bufs=1) as rp,
            tc.tile_pool(name="work", bufs=6) as wp,
            tc.tile_pool(name="work1", bufs=1) as wp1,
            tc.tile_pool(name="ps_se", bufs=2, space="PSUM") as ps_se,
            tc.tile_pool(name="ps_s", bufs=2, space="PSUM") as ps_s,
            tc.tile_pool(name="ps_f1b", bufs=1, space="PSUM") as ps_f1b,
            tc.tile_pool(name="ps_num", bufs=2, space="PSUM") as ps_num,
            tc.tile_pool(name="ps_mix", bufs=1, space="PSUM") as ps_mix,
            tc.tile_pool(name="dram", bufs=1, space="DRAM") as dp,
        ):
            # ------- persistent SBUF tiles + input DMAs -------
            t_adj = pp.tile([P, JC, N], f8)
            tbp_cm = tc.tile_pool(name="tbase", bufs=1)
            tbp = tbp_cm.__enter__()
            t_base = tbp.tile([P, JC, IB], f32)
            adjww = pp.tile([P, JC, IB], f32)
            b1 = pp.tile([P, JC, IB], f8)
            h_aug = pp.tile([P, GROUPS, JC, 4, NHID + 1], f16)
            w4 = pp.tile([P, CC, GROUPS, 4 * NHID], f16)
            avec = pp.tile([NHID, 2 * NHEADS], f32)
            wout = pp.tile([P, 4, NCLS], f32)
            aovec = pp.tile([NCLS, 2], f32)
            ident = pp.tile([P, P], f32)
            sel8 = pp.tile([NHEADS, NHEADS * P], f32)
            va_sb = pp.tile([P, 32], f16)
            f1sb = pp.tile([NHEADS, IB], f32)
            f2sb = pp.tile([P, P], f32)
            colsum = pp.tile([NCLS, 1], f32)
            ones_row = pp.tile([1, P], f32)     # all-ones row for broadcasts
            ones2 = pp.tile([P, P], f32)        # ones; sliced at partition 32/64 for Z broadcasts
            ones_col = pp.tile([P, 1], f32)
            f1osb = pp.tile([1, IB], f32)
            f2osb = pp.tile([P, JC], f32)
            hout_aug = pp.tile([P, JC, 33], f16)  # cols 0-15 h_out, 32 ones (Z row at aligned partition 32)
            oo = pp.tile([NCLS, IB], f32)
            out_sb = pp.tile([P, 4, NCLS], f32)
            # recycled slots: tiles whose lifetimes do not overlap
            t_adjc = rp.tile([P, JC, IB], f8, tag="early")
            wst = rp.tile([NHID, NHEADS * CC, P], f32, tag="mid")  # dead after va MMs
            xt = rp.tile([P, CC, N], f16, tag="mid")
            xtc = rp.tile([P, CC, IB], f16, tag="early2")

            for mc in range(JC):
                nc.sync.dma_start(out=t_adj[:, mc, :], in_=d_tadj[:, mc, :])
            for mc in range(0, JC, 4):
                nc.sync.dma_start(out=t_adjc[:, mc : mc + 4, :], in_=d_tadjc[:, mc : mc + 4, :])
                nc.sync.dma_start(out=t_base[:, mc : mc + 4, :], in_=d_tbase[:, mc : mc + 4, :])
            nc.sync.dma_start(out=wst[:], in_=d_wst[:])
            nc.sync.dma_start(out=avec[:], in_=d_avec[:])
            nc.sync.dma_start(out=xt[:, 0, :], in_=d_xt[:, 0, :])
            nc.sync.dma_start(out=xt[:, 1, :], in_=d_xt[:, 1, :])
            nc.sync.dma_start(out=xtc[:], in_=d_xtc[:])
            nc.sync.dma_start(out=w4[:], in_=d_w4[:])
            nc.sync.dma_start(out=wout[:], in_=d_wout[:])
            nc.sync.dma_start(out=aovec[:], in_=d_aovec[:])
            nc.sync.dma_start(out=ident[:], in_=d_ident[:])
            nc.sync.dma_start(out=sel8[:], in_=d_sel8[:])

            nc.gpsimd.memset(ones_row[:], 1.0)
            nc.gpsimd.memset(ones2[:], 1.0)
            nc.gpsimd.memset(ones_col[:], 1.0)
            nc.gpsimd.memset(h_aug[:], 1.0)      # ones column preset
            nc.gpsimd.memset(hout_aug[:], 0.0)
            nc.gpsimd.memset(hout_aug[:, :, 32:33], 1.0)

            # ------- phase B: va vectors, f1/f2, colsum -------
            va_ps = ps_mix.tile([P, 32], f32, tag="mix")
            for h in range(NHEADS):
                for cc in range(CC):
                    for k in range(2):
                        nc.tensor.matmul(
                            va_ps[:, (cc * 2 + k) * NHEADS + h : (cc * 2 + k) * NHEADS + h + 1],
                            wst[:, h * CC + cc, :],
                            avec[:, h * 2 + k : h * 2 + k + 1],
                        )
            # w3 folded into the va vectors => scores come out pre-scaled
            nc.scalar.activation(va_sb[:], va_ps[:], AF.Copy, scale=W3)

            cs_ps = ps_mix.tile([NCLS, 1], f32, tag="mix")
            for k in range(4):
                nc.tensor.matmul(cs_ps[:], wout[:, k, :], ones_col[:], start=(k == 0), stop=(k == 3))
            nc.scalar.copy(colsum[:], cs_ps[:])

            f1_ps = ps_mix.tile([NHEADS, IB], f32, tag="mix")
            for cc in range(CC):
                nc.tensor.matmul(
                    f1_ps[:], va_sb[:, cc * 16 : cc * 16 + 8], xtc[:, cc, :],
                    start=(cc == 0), stop=(cc == 1),
                )
            nc.scalar.copy(f1sb[:], f1_ps[:])

            f2_ps = ps_mix.tile([P, P], f32, tag="mix")
            for jc in range(JC):
                for cc in range(CC):
                    nc.tensor.matmul(
                        f2_ps[:, jc * 8 : jc * 8 + 8],
                        xt[:, cc, jc * P : (jc + 1) * P],
                        va_sb[:, cc * 16 + 8 : cc * 16 + 16],
                        start=(cc == 0), stop=(cc == 1),
                    )
            nc.scalar.copy(f2sb[:], f2_ps[:])

            # ------- phase C: h packing (h_aug[j, 65] per head) -------
            for g in range(GROUPS):
                for mc in range(JC):
                    hp_ps = ps_mix.tile([P, 4 * NHID], f32, tag="mix")
                    for cc in range(CC):
                        nc.tensor.matmul(
                            hp_ps[:], xt[:, cc, mc * P : (mc + 1) * P], w4[:, cc, g, :],
                            start=(cc == 0), stop=(cc == 1),
                        )
                    nc.scalar.copy(
                        h_aug[:, g, mc, :, 0:NHID],
                        hp_ps[:].rearrange("p (hh f) -> p hh f", hh=4),
                    )

            # ------- phase D: init_weight_graph -------
            for jc in range(JC):
                e1 = ps_se.tile([P, IB], f32, tag="se")
                for kp in range(JC // 2):
                    nc.tensor.matmul(
                        e1[:],
                        t_adj[:, 2 * kp : 2 * kp + 2, jc * P : (jc + 1) * P],
                        t_adjc[:, 2 * kp : 2 * kp + 2, :],
                        start=(kp == 0), stop=(kp == JC // 2 - 1),
                        perf_mode=mybir.MatmulPerfMode.DoubleRow,
                    )
                nc.scalar.activation(b1[:, jc, :], e1[:], AF.Sign)
                # adjww (partial) = r2*min(e1,1) + t_base
                nc.vector._custom_dve(
                    MSAB, out=adjww[:, jc, :], in0=e1[:], in1=t_base[:, jc, :],
                    s0=R2, s1=0.0,
                )
            for jc in range(JC):
                e2 = ps_se.tile([P, IB], f32, tag="se")
                for kp in range(JC // 2):
                    nc.tensor.matmul(
                        e2[:],
                        t_adj[:, 2 * kp : 2 * kp + 2, jc * P : (jc + 1) * P],
                        b1[:, 2 * kp : 2 * kp + 2, :],
                        start=(kp == 0), stop=(kp == JC // 2 - 1),
                        perf_mode=mybir.MatmulPerfMode.DoubleRow,
                    )
                # adjww = (1+BIG)*min(e2,1) + partial - BIG   (in-place in1)
                nc.vector._custom_dve(
                    MSAB, out=adjww[:, jc, :], in0=e2[:], in1=adjww[:, jc, :],
                    s0=1.0 + BIG, s1=-BIG,
                )

            if DBG:
                nc.sync.dma_start(out=d_dbg_adjww[:], in_=adjww[:])
                nc.sync.dma_start(out=d_dbg_f1[:], in_=f1sb[:])
                nc.sync.dma_start(out=d_dbg_f2[:], in_=f2sb[:])
                nc.sync.dma_start(out=d_dbg_haug[:], in_=h_aug[:])

            # ------- phase E: attention heads -------
            xh1 = rp.tile([P, 4, IB], f32, tag="mid")  # reuses xt slot
            cc_in = []
            cc_out = []
            for k in range(4):
                cci = dp.tile([NCLS, IB], f32, tag=f"ccin{k}", name=f"cc_in{k}")
                cco = dp.tile([4, NCLS, IB], f32, tag=f"ccout{k}", name=f"cc_out{k}")
                cc_in.append(cci)
                cc_out.append(cco)
            for h in range(NHEADS):
                g, hh = h // 4, h % 4
                f1b = ps_f1b.tile([P, IB], f32, tag="f1b")
                nc.tensor.matmul(f1b[:], sel8[:, h * P : (h + 1) * P], f1sb[:])
                f1bs = wp.tile([P, IB], f32, tag="f1bs")
                nc.scalar.copy(f1bs[:], f1b[:])
                num = ps_num.tile([NHID + 1, IB], f32, tag="num")
                for jc in range(JC):
                    s_ps = ps_s.tile([P, IB], f32, tag="s")
                    nc.vector._custom_dve(
                        LEAKY, out=s_ps[:], in0=f1bs[:], in1=adjww[:, jc, :],
                        s0=f2sb[:, jc * 8 + h : jc * 8 + h + 1], s1=NEGS, imm2=ALPHA,
                    )
                    p_sb = wp.tile([P, IB], bf16, tag="p")
                    nc.scalar.activation(p_sb[:], s_ps[:], AF.Exp)
                    nc.tensor.matmul(
                        num[:], h_aug[:, g, jc, hh, :], p_sb[:],
                        start=(jc == 0), stop=(jc == JC - 1),
                    )
                if h == 0:
                    tbp_cm.__exit__(None, None, None)
                    tlp_cm = tc.tile_pool(name="tails", bufs=3)
                    tlp = tlp_cm.__enter__()
                # tail: hp = num/Z ; xh1 = relu(hp) + exp(min(hp, 0))  (= elu+1)
                # Z is at partition 64; custom DVE ops need base partition 0:
                # ACT-copy out of PSUM then DMA down to partition 0.
                zq = tlp.tile([P, IB], f32, tag="zq")
                nc.scalar.copy(zq[64:65, :], num[NHID : NHID + 1, :])
                z0 = tlp.tile([1, IB], f32, tag="z0")
                nc.sync.dma_start(out=z0[:], in_=zq[64:65, :])
                nc.vector.reciprocal_approx_fast(zq[0:1, :], z0[:])
                rzb_ps = ps_mix.tile([NHID, IB], f32, tag="mix")
                nc.tensor.matmul(rzb_ps[:], ones_row[:, 0:NHID], zq[0:1, :])
                rzb = tlp.tile([NHID, IB], f32, tag="rzb")
                nc.scalar.copy(rzb[:], rzb_ps[:])
                hp = tlp.tile([NHID, IB], f32, tag="hp")
                nc.vector.scalar_tensor_tensor(
                    hp[:], num[0:NHID, :], 1.0, rzb[:], AL.mult, AL.mult
                )
                mm = zq[0:NHID, :]  # zq rows 0-63 free by now (row 0 read by rzb matmul)
                nc.vector.tensor_scalar_min(mm, hp[:], 0.0)
                ee = tlp.tile([NHID, IB], f32, tag="ee")
                nc.scalar.activation(ee[:], mm, AF.Exp)
                if h % 2 == 0:
                    nc.vector.scalar_tensor_tensor(
                        xh1[0:NHID, h // 2, :], hp[:], 0.0, ee[:], AL.max, AL.add,
                    )
                else:
                    tmp64 = tlp.tile([NHID, IB], f32, tag="tmp64")
                    nc.vector.scalar_tensor_tensor(
                        tmp64[:], hp[:], 0.0, ee[:], AL.max, AL.add,
                    )
                    nc.sync.dma_start(out=xh1[NHID : 2 * NHID, h // 2, :], in_=tmp64[:])
                    # pipelined partial h_out + AllGather for this head pair
                    k = h // 2
                    po_ps = ps_mix.tile([NCLS, IB], f32, tag="mix")
                    nc.tensor.matmul(po_ps[:], wout[:, k, :], xh1[:, k, :])
                    pol = tlp.tile([NCLS, IB], f32, tag="pol")
                    if k == 0:
                        # fold the elu+1 offset correction into partial 0
                        nc.vector.tensor_scalar_sub(pol[:], po_ps[:], colsum[:, 0:1])
                    else:
                        nc.scalar.copy(pol[:], po_ps[:])
                    nc.sync.dma_start(out=cc_in[k][:], in_=pol[:])
                    nc.gpsimd.collective_compute(
                        "AllGather",
                        mybir.AluOpType.bypass,
                        replica_groups=[[0, 1, 2, 3], [4, 5, 6, 7]],
                        ins=[cc_in[k].opt()],
                        outs=[cc_out[k].opt()],
                    )

            # ------- phase F: output layer -------
            # sum the 4 gathered partials into houtT
            houtT = rp.tile([NCLS, N], f32, tag="early")  # reuses t_adjc slot
            for r in range(4):
                nc.sync.dma_start(out=houtT[:, r * IB : (r + 1) * IB], in_=cc_out[0][r])
            for k in range(1, 4):
                gk = tlp.tile([NCLS, N], f32, tag="gk")
                for r in range(4):
                    nc.sync.dma_start(out=gk[:, r * IB : (r + 1) * IB], in_=cc_out[k][r])
                nc.vector.tensor_tensor(houtT[:], houtT[:], gk[:], AL.add)
            # local h_out slice for f1o (cheap recompute)
            ho_ps = ps_mix.tile([NCLS, IB], f32, tag="mix")
            for k in range(4):
                nc.tensor.matmul(ho_ps[:], wout[:, k, :], xh1[:, k, :], start=(k == 0), stop=(k == 3))
            hout_loc = tlp.tile([NCLS, IB], f32, tag="pol")
            nc.vector.tensor_scalar_sub(hout_loc[:], ho_ps[:], colsum[:, 0:1])
            if DBG:
                nc.sync.dma_start(out=d_dbg_xh1[:], in_=xh1[:])
                nc.sync.dma_start(out=d_dbg_hloc[:], in_=hout_loc[:])
                nc.sync.dma_start(out=d_dbg_hT[:], in_=houtT[:])

            # transpose houtT into hout_aug [j, 33]
            for jc in range(JC):
                tp_ps = ps_mix.tile([P, NCLS], f32, tag="mix")
                nc.tensor.transpose(tp_ps[:], houtT[:, jc * P : (jc + 1) * P], ident[0:NCLS, 0:NCLS])
                nc.scalar.copy(hout_aug[:, jc, 0:NCLS], tp_ps[:])

            # f2o columns; f1o row from local slice
            f2o_ps = ps_mix.tile([P, JC], f32, tag="mix")
            for jc in range(JC):
                nc.tensor.matmul(
                    f2o_ps[:, jc : jc + 1], houtT[:, jc * P : (jc + 1) * P], aovec[:, 1:2]
                )
            nc.scalar.activation(f2osb[:], f2o_ps[:], AF.Copy, scale=W3)
            f1o_ps = ps_mix.tile([1, IB], f32, tag="mix")
            nc.tensor.matmul(f1o_ps[:], aovec[:, 0:1], hout_loc[:])
            nc.scalar.activation(f1osb[:], f1o_ps[:], AF.Copy, scale=W3)

            f1bo = ps_f1b.tile([P, IB], f32, tag="f1b")
            nc.tensor.matmul(f1bo[:], ones_row[:], f1osb[:])
            f1bos = wp.tile([P, IB], f32, tag="f1bs")
            nc.scalar.copy(f1bos[:], f1bo[:])
            onum = ps_num.tile([33, IB], f32, tag="num")
            for jc in range(JC):
                s_ps = ps_s.tile([P, IB], f32, tag="s")
                nc.vector._custom_dve(
                    LEAKY, out=s_ps[:], in0=f1bos[:], in1=adjww[:, jc, :],
                    s0=f2osb[:, jc : jc + 1], s1=NEGS, imm2=ALPHA,
                )
                p_sb = wp.tile([P, IB], bf16, tag="p")
                nc.scalar.activation(p_sb[:], s_ps[:], AF.Exp)
                nc.tensor.matmul(
                    onum[:], hout_aug[:, jc, :], p_sb[:],
                    start=(jc == 0), stop=(jc == JC - 1),
                )
            # out-layer tail (Z at partition 32)
            zq = tlp.tile([P, IB], f32, tag="zq")
            nc.scalar.copy(zq[32:33, :], onum[32:33, :])
            z0 = tlp.tile([1, IB], f32, tag="z0")
            nc.sync.dma_start(out=z0[:], in_=zq[32:33, :])
            nc.vector.reciprocal_approx_fast(zq[0:1, :], z0[:])
            rzb_ps = ps_mix.tile([NCLS, IB], f32, tag="mix")
            nc.tensor.matmul(rzb_ps[:], ones_row[:, 0:NCLS], zq[0:1, :])
            rzb = tlp.tile([NHID, IB], f32, tag="rzb")
            nc.scalar.copy(rzb[0:NCLS, :], rzb_ps[:])
            hp = tlp.tile([NHID, IB], f32, tag="hp")
            nc.vector.scalar_tensor_tensor(
                hp[0:NCLS, :], onum[0:NCLS, :], 1.0, rzb[0:NCLS, :], AL.mult, AL.mult
            )
            mm = zq[0:NCLS, :]
            nc.vector.tensor_scalar_min(mm, hp[0:NCLS, :], 0.0)
            ee = tlp.tile([NHID, IB], f32, tag="ee")
            nc.scalar.activation(ee[0:NCLS, :], mm, AF.Exp)
            oo_raw = tlp.tile([NCLS, IB], f32, tag="tmp64")
            nc.vector.scalar_tensor_tensor(
                oo_raw[:], hp[0:NCLS, :], 0.0, ee[0:NCLS, :], AL.max, AL.add
            )
            nc.vector.tensor_scalar_sub(oo[:], oo_raw[:], 1.0)

            # transpose [16, 512] -> [512, 16] and store
            for k in range(4):
                to_ps = ps_mix.tile([P, NCLS], f32, tag="mix")
                nc.tensor.transpose(to_ps[:], oo[:, k * P : (k + 1) * P], ident[0:NCLS, 0:NCLS])
                nc.scalar.copy(out_sb[:, k, :], to_ps[:])
                nc.sync.dma_start(out=d_out[k * P : (k + 1) * P, :], in_=out_sb[:, k, :])
            tlp_cm.__exit__(None, None, None)

    nc.compile()
    _CACHE["nc"] = nc
    return nc


# --------------------------------------------------------------------------- #
# host side
# --------------------------------------------------------------------------- #
def _prep_core_inputs(x, adj, Ws, As, W_out, a_out):
    """Build the per-core input dicts (host-side sharding / layout prep)."""
    eye = np.eye(N, dtype=np.float32)
    # weights (shared by all cores)
    w4 = (
        Ws.transpose(1, 0, 2).reshape(NFEAT, NHEADS * NHID)
        .reshape(CC, P, GROUPS, 4 * NHID).transpose(1, 0, 2, 3)
    ).astype(np.float16).copy()
    wst = Ws.transpose(2, 0, 1).reshape(NHID, NHEADS, CC, P).reshape(
        NHID, NHEADS * CC, P
    ).astype(np.float32).copy()
    avec = (
        As[:, :, 0].reshape(NHEADS, 2, NHID).transpose(2, 0, 1).reshape(NHID, 2 * NHEADS)
    ).astype(np.float32).copy()
    wout = W_out.reshape(4, P, NCLS).transpose(1, 0, 2).astype(np.float32).copy()
    aovec = a_out[:, 0].reshape(2, NCLS).T.astype(np.float32).copy()
    ident = np.eye(P, dtype=np.float32)
    sel8 = np.zeros((NHEADS, NHEADS * P), np.float32)
    for h in range(NHEADS):
        sel8[h, h * P : (h + 1) * P] = 1.0

    shared = dict(w4=w4, wst=wst, avec=avec, wout=wout, aovec=aovec, ident=ident, sel8=sel8)

    in_maps = []
    for b in range(B):
        adjT = np.ascontiguousarray(adj[b].T)
        t_adj_full = adjT.reshape(JC, P, N).transpose(1, 0, 2).astype(F8NP).copy()
        base_full = (R1 * adjT + R0 * eye).astype(np.float32)
        xT = np.ascontiguousarray(x[b].T.astype(np.float16))
        xt_full = xT.reshape(CC, P, N).transpose(1, 0, 2).copy()
        for s in range(4):
            i0 = s * IB
            t_adjc = (
                adjT[:, i0 : i0 + IB].reshape(JC, P, IB).transpose(1, 0, 2).astype(F8NP).copy()
            )
            t_base = (
                base_full[:, i0 : i0 + IB].reshape(JC, P, IB).transpose(1, 0, 2).copy()
            )
            xtc = xt_full[:, :, i0 : i0 + IB].copy()
            in_maps.append(
                dict(
                    t_adj=t_adj_full, t_adjc=t_adjc, t_base=t_base,
                    xt=xt_full, xtc=xtc, **shared,
                )
            )
    return in_maps


def kernel(x, adj, Ws, As, W_out, a_out, d_window):
    assert int(d_window) == 3, f"kernel hardcodes d_window=3, got {d_window}"
    x = np.asarray(x, np.float32)
    adj = np.asarray(adj, np.float32)
    Ws = np.asarray(Ws, np.float32)
    As = np.asarray(As, np.float32)
    W_out = np.asarray(W_out, np.float32)
    a_out = np.asarray(a_out, np.float32)

    from concourse import bass_utils

    nc = _build_nc()
    in_maps = _prep_core_inputs(x, adj, Ws, As, W_out, a_out)
    res = bass_utils.run_bass_kernel_spmd(nc, in_maps, core_ids=list(range(NCORES)))
    _CACHE["last_results"] = res

    out = np.zeros((B, N, NCLS), np.float32)
    for c in range(NCORES):
        b, s = c // 4, c % 4
        out[b, s * IB : (s + 1) * IB, :] = res.results[c]["out"]
    return out


if __name__ == "__main__":
    import reference

    inputs = reference.setup_inputs()
    inputs = {k: np.asarray(v) for k, v in inputs.items()}
    expected = np.asarray(reference.reference(**inputs))
    actual = kernel(**inputs)
    err = np.abs(actual - expected).max() / np.abs(expected).max()
    print("Relative error:", err)
